# revision 21
# baseline (speedup 1.0000x reference)
"""Trainium2 Bass kernel for nn_Attention_2826088481156 (v2).

Dense transformer attention block:
    qkv = x @ W_qkv.T + b_qkv            [B,T,3,H,D]
    scores = q k^T * SCALE + log(clip(cutoffs, 1e-15))
    attn = softmax(scores)
    out  = (attn @ v) @ W_out.T + b_out

Sharding (8 NeuronCores): data-parallel over B (=2), tensor-parallel over
heads (16 heads -> 4 per core).  Each core computes the full attention for
its 4 heads and a partial output projection over its 256 channels; the
host sums the 4 partials per batch and adds the (host-folded) biases.

Key algebra on device:
    softmax(s + log c) = (c * exp(s)) / sum_k(c * exp(s))   [no log, no max]
    attn @ [V | 1] gives the weighted values and the softmax denominator
    (row 64 of the PSUM accumulator) in one pass.
    b_v and b_out never enter the nonlinearity; host folds y += W_out@b_v + b_out.
    sqrt(1/8) scaling folded into W_q/W_k/b_q/b_k host-side.

v2 structural changes vs the 388us v1 baseline (engine-level findings from
real NTFF traces):
  * ACT (exp over 16.8M scores/core) is the 141us floor; everything else
    must hide underneath it, and the PE must stay busy to hold its 2.4GHz
    p-state (idle gaps drop it to 1.2GHz - measured 427ns vs 216ns per
    512-col matmul).
  * P = E * cutoffs alternates between DVE and the otherwise-idle
    Pool/GPSIMD engine (DVE was 194us busy in v1; fp8 everywhere was
    tried and rejected: absmax error 2-6% vs the 2% budget).
  * v1 spent 52us in 8 single-partition DVE RECIPROCALs; the softmax
    denominators are now DMA'd into one [4, QW] staging tile per query
    chunk -> ONE batched reciprocal (engines can only address partition
    bases 0/32/64/96, so rows are re-staged to partition 0 by tiny DMAs
    before the gpsimd broadcast).
  * attention-output PSUM is evacuated by a fast copy (psO runs with a
    single buffer) and the out-projection gets a dedicated PSUM bank.
  * stage B (V projection), most of stage A, and the qcc0 out-projection
    are emitted as PE filler work inside the attention pipeline so the
    in-order PE stream always has independent work while ACT chews
    (keeps the p-state up AND hides ~60us of projection work).
"""

import numpy as np

import concourse.bass as bass
import concourse.tile as tile
from concourse import bacc, mybir
from concourse.bass_utils import run_bass_kernel_spmd
from concourse.bass_interp import get_hw_module

F16 = mybir.dt.float16
F32 = mybir.dt.float32
Exp = mybir.ActivationFunctionType.Exp

DIM = 1024
H = 16
D = 64
B = 2
T = 2048
RSCALE = 0.125 ** 0.5   # folded into W_q, W_k, b_q, b_k on host
HPC = 4                 # heads per core
CH = HPC * D            # 256 channels per core
NCORES = 8

_cache = {}


def build_kernel(t=T, compile_hw=True, loop_reps=0, phase=5, opt=None):
    import os
    from collections import deque
    from contextlib import ExitStack, nullcontext
    _opt = dict(opt or {})
    for k in ("LAG", "FILLROWS", "SERIAL_AB", "PMODE"):
        if f"K_{k}" in os.environ:
            _opt.setdefault(k, int(os.environ[f"K_{k}"]))
    LAG = int(_opt.get("LAG", 3))        # attnV trails scores by LAG kb tiles
    FILLROWS = int(_opt.get("FILLROWS", 2560))  # PE filler rows per step
    SERIAL_AB = int(_opt.get("SERIAL_AB", 0))
    PMODE = int(_opt.get("PMODE", 3))    # P-mult: 0=DVE, 1=Pool, 2=50/50, 3=25% Pool

    nc = bacc.Bacc("TRN2", target_bir_lowering=False, debug=False,
                   num_devices=NCORES)

    n_cb = DIM // 128           # 8 contraction blocks for projections
    n_kb = t // 128             # 16 key blocks
    QW = 1024 if t >= 1024 else t
    n_qcc = t // QW
    n_t2 = t // 512             # 512-token chunks for stage A

    xT = nc.dram_tensor("xT", [DIM, t], F16, kind="ExternalInput")
    cT = nc.dram_tensor("cT", [t, t], F16, kind="ExternalInput")
    wqkT = nc.dram_tensor("wqkT", [DIM, 512], F16, kind="ExternalInput")
    wvT = nc.dram_tensor("wvT", [DIM, CH], F16, kind="ExternalInput")
    woT = nc.dram_tensor("woT", [CH, DIM], F16, kind="ExternalInput")
    bqk = nc.dram_tensor("bqk", [128, 4], F32, kind="ExternalInput")
    yT = nc.dram_tensor("yT", [DIM, t], F16, kind="ExternalOutput")

    with tile.TileContext(nc) as tc:
        loop_ctx = tc.For_i(0, loop_reps, 1) if loop_reps else nullcontext()
        with loop_ctx, ExitStack() as ctx:
            const = ctx.enter_context(tc.tile_pool(name="const", bufs=1))
            qkp = ctx.enter_context(tc.tile_pool(name="qkT", bufs=1))
            vp = ctx.enter_context(tc.tile_pool(name="v65", bufs=1))
            otp = ctx.enter_context(tc.tile_pool(name="ot", bufs=1))
            xp = ctx.enter_context(tc.tile_pool(name="xTp", bufs=1))

            wqk_sb = []
            for cb in range(n_cb):
                w1 = const.tile([128, 512], F16, tag=f"wqk{cb}", name=f"wqk{cb}")
                nc.sync.dma_start(w1[:], wqkT[cb * 128:(cb + 1) * 128, :])
                wqk_sb.append(w1)
            xT_sb = [xp.tile([128, t], F16, tag=f"x{cb}", name=f"xt{cb}")
                     for cb in range(n_cb)]
            for tb2 in range(n_t2):
                for cb in range(n_cb):
                    nc.sync.dma_start(
                        xT_sb[cb][:, tb2 * 512:(tb2 + 1) * 512],
                        xT[cb * 128:(cb + 1) * 128, tb2 * 512:(tb2 + 1) * 512])
            wv_sb = []
            for cb in range(n_cb):
                w2 = const.tile([128, CH], F16, tag=f"wv{cb}", name=f"wv{cb}")
                nc.sync.dma_start(w2[:], wvT[cb * 128:(cb + 1) * 128, :])
                wv_sb.append(w2)
            wo_sb = []
            for j in range(2):
                w3 = const.tile([128, DIM], F16, tag=f"wo{j}", name=f"wo{j}")
                nc.sync.dma_start(w3[:], woT[j * 128:(j + 1) * 128, :])
                wo_sb.append(w3)
            bqk_sb = const.tile([128, 4], F32, tag="bqk")
            nc.sync.dma_start(bqk_sb[:], bqk[:, :])

            # qkT_sb[j]: j=0 Q heads 0-1, j=1 Q heads 2-3, j=2/3 same for K
            qkT_sb = [qkp.tile([128, t], F16, tag=f"qk{j}", name=f"qkT{j}")
                      for j in range(4)]
            # v65_sb[tb][:, h, 0:64] = V head h rows tb; [:, h, 64] = 1.0
            v65_sb = [vp.tile([128, HPC, 65], F16, tag=f"v{tb}", name=f"v65_{tb}")
                      for tb in range(n_kb)]
            # normalized attention output^T: ot_sb[j] heads (2j, 2j+1)
            ot_sb = [otp.tile([128, t], F16, tag=f"ot{j}", name=f"ot{j}")
                     for j in range(2)]
            # transposed denominator staging: [128, 8] per head, so the
            # reciprocal runs over 8 elements/partition instead of 1024


            # ---- Stage A chunks: qk^T = W_qk @ x^T (+bias) ----
            a_done = [0]

            def emit_a(ob, tb2, pool):
                pa = pool.tile([128, 512], F32, tag="pab", name="pa")
                for cb in range(n_cb):
                    nc.tensor.matmul(
                        pa[:], wqk_sb[cb][:, ob * 128:(ob + 1) * 128],
                        xT_sb[cb][:, tb2 * 512:(tb2 + 1) * 512],
                        start=(cb == 0), stop=(cb == n_cb - 1))
                nc.vector.tensor_scalar_add(
                    qkT_sb[ob][:, tb2 * 512:(tb2 + 1) * 512],
                    pa[:], bqk_sb[:, ob:ob + 1])
                a_done[0] += 1

            # ---- Stage B chunks: V = x @ W_v^T ----
            b_done = [0]

            def emit_b(tb, pool):
                pt = pool.tile([128, 512], F32, tag="pab", name="pb")
                pb = pt[:, 0:CH]
                for cb in range(n_cb):
                    nc.tensor.matmul(
                        pb, xT_sb[cb][:, tb * 128:(tb + 1) * 128],
                        wv_sb[cb][:], start=(cb == 0), stop=(cb == n_cb - 1))
                nc.vector.memset(v65_sb[tb][:, :, 64:65], 1.0)
                nc.vector.tensor_copy(
                    v65_sb[tb][:, :, 0:64],
                    pb.rearrange("p (h d) -> p h d", d=D))
                b_done[0] += 1

            # serial prefix: q heads 0-1 and k heads 0-1 for the first 1024
            # tokens (everything the first scores tiles touch); the rest of
            # stage A and all of stage B interleave into the pipeline.
            n_pre = min(2, n_t2)   # 512-chunks covering the first q-chunk
            if phase >= 1:
                with tc.tile_pool(name="psPre", bufs=2, space="PSUM") as psPre:
                    emit_a(0, 0, psPre)
                    emit_a(2, 0, psPre)
                    if n_pre > 1:
                        emit_a(0, 1, psPre)
            fillers = deque()
            if phase >= 1:
                # pop order tuned so each chunk lands before its first use:
                # B(tb) needed at step tb+LAG of (qcc0,h0); k-chunks (ob2)
                # tb2=2,3 by steps 8/12; everything else has lots of slack.
                early = [("b", tb) for tb in range(min(4, n_kb))]
                if n_pre > 1:
                    early.insert(4, ("a", 2, 1))
                if n_t2 > 2:
                    early.append(("a", 2, 2))
                early += [("b", 4), ("b", 5)] if n_kb > 5 else []
                if n_t2 > 3:
                    early.append(("a", 2, 3))
                late = [("b", tb) for tb in range(6, n_kb)]
                late += [("a", 1, tb2) for tb2 in range(n_t2)]
                late += [("a", 3, tb2) for tb2 in range(n_t2)]
                late += [("a", 0, tb2) for tb2 in range(n_pre, n_t2)]
                late += [("a", 2, tb2) for tb2 in range(4, n_t2)]
                if phase < 2:
                    early = [it for it in early if it[0] != "b"]
                    late = [it for it in late if it[0] != "b"]
                for item in early + late:
                    if item[0] == "b" and item[1] >= n_kb:
                        continue
                    fillers.append(item)
            if (phase < 3 or SERIAL_AB) and fillers:
                with tc.tile_pool(name="psPre2", bufs=2, space="PSUM") as psPre2:
                    while fillers:
                        it = fillers.popleft()
                        if it[0] == "b":
                            emit_b(it[1], psPre2)
                        else:
                            emit_a(it[1], it[2], psPre2)

            n_qcc_eff = n_qcc if phase >= 3 else 0
            psY_ctx = ExitStack()
            psY = [None]

            # ---- Stage C: attention + output projection ----
            with tc.tile_pool(name="cTp", bufs=18) as cp, \
                 tc.tile_pool(name="ep", bufs=6) as ep, \
                 tc.tile_pool(name="pp", bufs=LAG + 2) as pp, \
                 tc.tile_pool(name="orp", bufs=3) as orp, \
                 tc.tile_pool(name="rbp", bufs=3) as rbp, \
                 tc.tile_pool(name="tmpp", bufs=3) as tmpp, \
                 tc.tile_pool(name="ysp", bufs=4) as ysp, \
                 tc.tile_pool(name="psS", bufs=2, space="PSUM") as psS, \
                 tc.tile_pool(name="psO", bufs=1, space="PSUM") as psO:

                psAB_ctx = ExitStack()
                psAB = [None]
                if fillers and n_qcc_eff:
                    psAB[0] = psAB_ctx.enter_context(
                        tc.tile_pool(name="psAB", bufs=2, space="PSUM"))

                def emit_outproj(qcc, ob, ns, use_scalar=False):
                    if psY[0] is None:
                        # psAB banks are free by now (all A/B chunks precede
                        # any out-projection in the filler queue)
                        psAB_ctx.close()
                        psY[0] = psY_ctx.enter_context(
                            tc.tile_pool(name="psY", bufs=2, space="PSUM"))
                    q0 = qcc * QW + ns * 512
                    Y = psY[0].tile([128, 512], F32, tag="Y", name="Y")
                    for cb in range(2):
                        nc.tensor.matmul(
                            Y[:], wo_sb[cb][:, ob * 128:(ob + 1) * 128],
                            ot_sb[cb][:, q0:q0 + 512],
                            start=(cb == 0), stop=(cb == 1))
                    ys = ysp.tile([128, 512], F16, tag="ys", name="ys")
                    if use_scalar:
                        nc.scalar.copy(ys[:], Y[:])
                    else:
                        nc.vector.tensor_copy(ys[:], Y[:])
                    nc.sync.dma_start(yT[ob * 128:(ob + 1) * 128, q0:q0 + 512],
                                      ys[:])

                def pop_fillers(budget, y_ok=True):
                    while fillers and budget > 0:
                        it = fillers[0]
                        if it[0] == "y" and not y_ok:
                            return
                        fillers.popleft()
                        if it[0] == "b":
                            emit_b(it[1], psAB[0])
                            budget -= 2048
                        elif it[0] == "a":
                            emit_a(it[1], it[2], psAB[0])
                            budget -= 4096
                        else:
                            emit_outproj(it[1], it[2], it[3])
                            budget = 0

                for qcc in range(n_qcc_eff):
                    q0 = qcc * QW
                    cT_tiles = []
                    for kb in range(n_kb):
                        ct = cp.tile([128, QW], F16, tag="ct", name=f"ct{kb}")
                        nc.sync.dma_start(ct[:],
                                          cT[kb * 128:(kb + 1) * 128, q0:q0 + QW])
                        cT_tiles.append(ct)

                    for h in range(HPC):
                        j, bp = h // 2, (h % 2) * 64
                        O = psO.tile([65, QW], F32, tag="O", name="O")
                        p_tiles = {}

                        def emit_attnv(kb):
                            P = p_tiles.pop(kb)
                            for ns in range(QW // 512):
                                nc.tensor.matmul(
                                    O[:, ns * 512:(ns + 1) * 512],
                                    v65_sb[kb][:, h, :],
                                    P[:, ns * 512:(ns + 1) * 512],
                                    start=(kb == 0), stop=(kb == n_kb - 1))

                        for i in range(n_kb):
                            if i >= LAG and phase >= 4:
                                emit_attnv(i - LAG)
                            S = psS.tile([128, QW], F32, tag="S", name="S")
                            for ns in range(QW // 512):
                                nc.tensor.matmul(
                                    S[:, ns * 512:(ns + 1) * 512],
                                    qkT_sb[2 + j][bp:bp + 64, i * 128:(i + 1) * 128],
                                    qkT_sb[j][bp:bp + 64,
                                              q0 + ns * 512:q0 + (ns + 1) * 512],
                                    start=True, stop=True)
                            E = ep.tile([128, QW], F16, tag="E", name="E")
                            nc.scalar.activation(E[:], S[:], Exp, scale=1.0)
                            P = pp.tile([128, QW], F16, tag="P", name="P")
                            p_tiles[i] = P
                            use_pool = (PMODE == 1 or
                                        (PMODE == 2 and i % 2 == 1) or
                                        (PMODE == 3 and i % 4 == 3))
                            eng = nc.gpsimd if use_pool else nc.vector
                            eng.tensor_mul(P[:], E[:], cT_tiles[i][:])
                            pop_fillers(FILLROWS, y_ok=(h > 0 or i >= 4))
                        if phase >= 4:
                            for kb in range(n_kb - LAG, n_kb):
                                emit_attnv(kb)
                        if phase < 4:
                            continue
                        # per-head normalization, fully pipelined: fast
                        # approx reciprocal (fp32, ~18 bits) of the
                        # denominator row, gpsimd-broadcast, multiply.
                        den32 = rbp.tile([1, QW], F32, tag="den32", name="den32")
                        nc.vector.tensor_copy(den32[:], O[64:65, :])
                        oraw = orp.tile([64, QW], F16, tag="oraw", name="oraw")
                        nc.vector.tensor_copy(oraw[:], O[0:64, :])
                        rr32 = rbp.tile([1, QW], F32, tag="rr32", name="rr32")
                        nc.vector.reciprocal_approx_fast(rr32[:], den32[:])
                        rb32 = rbp.tile([64, QW], F32, tag="rb32", name="rb32")
                        nc.gpsimd.partition_broadcast(rb32[:], rr32[:])
                        tmp = tmpp.tile([64, QW], F16, tag="tmp", name="tmp")
                        nc.vector.tensor_mul(tmp[:], oraw[:], rb32[:])
                        nc.sync.dma_start(ot_sb[j][bp:bp + 64, q0:q0 + QW],
                                          tmp[:])

                    if phase < 5:
                        continue
                    if qcc < n_qcc - 1:
                        for ob in range(8):
                            for ns in range(QW // 512):
                                fillers.append(("y", qcc, ob, ns))
                    else:
                        while fillers:
                            pop_filler()
                        for ob in range(8):
                            for ns in range(QW // 512):
                                emit_outproj(qcc, ob, ns,
                                             use_scalar=(ns == 0))
                psY_ctx.close()

    nc.compile()
    if compile_hw:
        nc.m = get_hw_module(nc.m)
    return nc


def make_in_maps(x, cutoffs, W_qkv, b_qkv, W_out):
    """Host-side sharding: slice + transpose + cast per core.

    wqkT column blocks: ob0 = Q heads {0,1}, ob1 = Q heads {2,3},
    ob2/ob3 = same for K; all scaled by sqrt(1/8)."""
    per_batch = []
    for b in range(B):
        xT_b = np.ascontiguousarray(x[b].T).astype(np.float16)
        cT_b = np.ascontiguousarray(cutoffs[b].T).astype(np.float16)
        per_batch.append((xT_b, cT_b))

    in_maps = []
    for core in range(NCORES):
        b, hg = core // HPC, core % HPC
        ch = slice(hg * CH, (hg + 1) * CH)
        chk = slice(DIM + hg * CH, DIM + (hg + 1) * CH)
        cols = [W_qkv[ch][:128], W_qkv[ch][128:],
                W_qkv[chk][:128], W_qkv[chk][128:]]
        wqkT_c = np.ascontiguousarray(
            (np.concatenate(cols, axis=0) * RSCALE).T).astype(np.float16)
        bv = np.concatenate([b_qkv[ch], b_qkv[chk]]) * RSCALE
        bqk_c = np.ascontiguousarray(
            np.stack([bv[0:128], bv[128:256], bv[256:384], bv[384:512]],
                     axis=1)).astype(np.float32)
        wvT_c = np.ascontiguousarray(
            W_qkv[2 * DIM + hg * CH:2 * DIM + (hg + 1) * CH, :].T).astype(np.float16)
        woT_c = np.ascontiguousarray(W_out[:, ch].T).astype(np.float16)
        in_maps.append({
            "xT": per_batch[b][0], "cT": per_batch[b][1],
            "wqkT": wqkT_c, "wvT": wvT_c, "woT": woT_c, "bqk": bqk_c,
        })
    return in_maps


def kernel(x, cutoffs, W_qkv, b_qkv, W_out, b_out):
    x = np.asarray(x, dtype=np.float32)
    cutoffs = np.asarray(cutoffs, dtype=np.float32)
    W_qkv = np.asarray(W_qkv, dtype=np.float32)
    b_qkv = np.asarray(b_qkv, dtype=np.float32)
    W_out = np.asarray(W_out, dtype=np.float32)
    b_out = np.asarray(b_out, dtype=np.float32)

    if "nc" not in _cache:
        _cache["nc"] = build_kernel()
    nc = _cache["nc"]

    in_maps = make_in_maps(x, cutoffs, W_qkv, b_qkv, W_out)
    res = None
    last_err = None
    for attempt in range(3):
        try:
            res = run_bass_kernel_spmd(nc, in_maps, core_ids=list(range(NCORES)),
                                       trace=False)
            break
        except Exception as e:  # transient NRT/axon failures: retry
            last_err = e
            import time
            time.sleep(5)
    if res is None:
        raise last_err

    y = np.zeros((B, T, DIM), dtype=np.float32)
    for core in range(NCORES):
        b = core // HPC
        y[b] += res.results[core]["yT"].astype(np.float32).T
    bias_vec = W_out @ b_qkv[2 * DIM:] + b_out
    y += bias_vec[None, None, :]
    return y


# revision 23
# speedup vs baseline: 1.0389x; 1.0389x over previous
"""Trainium2 Bass kernel for nn_Attention_2826088481156 (v2).

Dense transformer attention block:
    qkv = x @ W_qkv.T + b_qkv            [B,T,3,H,D]
    scores = q k^T * SCALE + log(clip(cutoffs, 1e-15))
    attn = softmax(scores)
    out  = (attn @ v) @ W_out.T + b_out

Sharding (8 NeuronCores): data-parallel over B (=2), tensor-parallel over
heads (16 heads -> 4 per core).  Each core computes the full attention for
its 4 heads and a partial output projection over its 256 channels; the
host sums the 4 partials per batch and adds the (host-folded) biases.

Key algebra on device:
    softmax(s + log c) = (c * exp(s)) / sum_k(c * exp(s))   [no log, no max]
    attn @ [V | 1] gives the weighted values and the softmax denominator
    (row 64 of the PSUM accumulator) in one pass.
    b_v and b_out never enter the nonlinearity; host folds y += W_out@b_v + b_out.
    sqrt(1/8) scaling folded into W_q/W_k/b_q/b_k host-side.

v2 structural changes vs the 388us v1 baseline (engine-level findings from
real NTFF traces):
  * ACT (exp over 16.8M scores/core) is the 141us floor; everything else
    must hide underneath it, and the PE must stay busy to hold its 2.4GHz
    p-state (idle gaps drop it to 1.2GHz - measured 427ns vs 216ns per
    512-col matmul).
  * P = E * cutoffs alternates between DVE and the otherwise-idle
    Pool/GPSIMD engine (DVE was 194us busy in v1; fp8 everywhere was
    tried and rejected: absmax error 2-6% vs the 2% budget).
  * v1 spent 52us in 8 single-partition DVE RECIPROCALs; the softmax
    denominators are now DMA'd into one [4, QW] staging tile per query
    chunk -> ONE batched reciprocal (engines can only address partition
    bases 0/32/64/96, so rows are re-staged to partition 0 by tiny DMAs
    before the gpsimd broadcast).
  * attention-output PSUM is evacuated by a fast copy (psO runs with a
    single buffer) and the out-projection gets a dedicated PSUM bank.
  * stage B (V projection), most of stage A, and the qcc0 out-projection
    are emitted as PE filler work inside the attention pipeline so the
    in-order PE stream always has independent work while ACT chews
    (keeps the p-state up AND hides ~60us of projection work).
"""

import numpy as np

import concourse.bass as bass
import concourse.tile as tile
from concourse import bacc, mybir
from concourse.bass_utils import run_bass_kernel_spmd
from concourse.bass_interp import get_hw_module

F16 = mybir.dt.float16
F32 = mybir.dt.float32
Exp = mybir.ActivationFunctionType.Exp

DIM = 1024
H = 16
D = 64
B = 2
T = 2048
RSCALE = 0.125 ** 0.5   # folded into W_q, W_k, b_q, b_k on host
HPC = 4                 # heads per core
CH = HPC * D            # 256 channels per core
NCORES = 8

_cache = {}


def build_kernel(t=T, compile_hw=True, loop_reps=0, phase=5, opt=None):
    import os
    from collections import deque
    from contextlib import ExitStack, nullcontext
    _opt = dict(opt or {})
    for k in ("LAGQ", "FILLROWS", "SERIAL_AB", "PMODE"):
        if f"K_{k}" in os.environ:
            _opt.setdefault(k, int(os.environ[f"K_{k}"]))
    LAGQ = int(_opt.get("LAGQ", 10))     # attnV trails scores by LAGQ steps
    FILLROWS = int(_opt.get("FILLROWS", 2560))  # PE filler rows per step
    SERIAL_AB = int(_opt.get("SERIAL_AB", 0))
    PMODE = int(_opt.get("PMODE", 3))    # P-mult: 0=DVE, 1=Pool, 2=50/50, 3=25% Pool

    nc = bacc.Bacc("TRN2", target_bir_lowering=False, debug=False,
                   num_devices=NCORES)

    n_cb = DIM // 128           # 8 contraction blocks for projections
    n_kb = t // 128             # 16 key blocks
    QW = 1024 if t >= 1024 else t
    n_qcc = t // QW
    n_t2 = t // 512             # 512-token chunks for stage A

    xT = nc.dram_tensor("xT", [DIM, t], F16, kind="ExternalInput")
    cT = nc.dram_tensor("cT", [t, t], F16, kind="ExternalInput")
    wqkT = nc.dram_tensor("wqkT", [DIM, 512], F16, kind="ExternalInput")
    wvT = nc.dram_tensor("wvT", [DIM, CH], F16, kind="ExternalInput")
    woT = nc.dram_tensor("woT", [CH, DIM], F16, kind="ExternalInput")
    bqk = nc.dram_tensor("bqk", [128, 4], F32, kind="ExternalInput")
    yT = nc.dram_tensor("yT", [DIM, t], F16, kind="ExternalOutput")

    with tile.TileContext(nc) as tc:
        loop_ctx = tc.For_i(0, loop_reps, 1) if loop_reps else nullcontext()
        with loop_ctx, ExitStack() as ctx:
            const = ctx.enter_context(tc.tile_pool(name="const", bufs=1))
            qkp = ctx.enter_context(tc.tile_pool(name="qkT", bufs=1))
            vp = ctx.enter_context(tc.tile_pool(name="v65", bufs=1))
            otp = ctx.enter_context(tc.tile_pool(name="ot", bufs=1))
            xp = ctx.enter_context(tc.tile_pool(name="xTp", bufs=1))

            wqk_sb = []
            for cb in range(n_cb):
                w1 = const.tile([128, 512], F16, tag=f"wqk{cb}", name=f"wqk{cb}")
                nc.sync.dma_start(w1[:], wqkT[cb * 128:(cb + 1) * 128, :])
                wqk_sb.append(w1)
            xT_sb = [xp.tile([128, t], F16, tag=f"x{cb}", name=f"xt{cb}")
                     for cb in range(n_cb)]
            for tb2 in range(n_t2):
                for cb in range(n_cb):
                    nc.sync.dma_start(
                        xT_sb[cb][:, tb2 * 512:(tb2 + 1) * 512],
                        xT[cb * 128:(cb + 1) * 128, tb2 * 512:(tb2 + 1) * 512])
            wv_sb = []
            for cb in range(n_cb):
                w2 = const.tile([128, CH], F16, tag=f"wv{cb}", name=f"wv{cb}")
                nc.sync.dma_start(w2[:], wvT[cb * 128:(cb + 1) * 128, :])
                wv_sb.append(w2)
            wo_sb = []
            for j in range(2):
                w3 = const.tile([128, DIM], F16, tag=f"wo{j}", name=f"wo{j}")
                nc.sync.dma_start(w3[:], woT[j * 128:(j + 1) * 128, :])
                wo_sb.append(w3)
            bqk_sb = const.tile([128, 4], F32, tag="bqk")
            nc.sync.dma_start(bqk_sb[:], bqk[:, :])

            # qkT_sb[j]: j=0 Q heads 0-1, j=1 Q heads 2-3, j=2/3 same for K
            qkT_sb = [qkp.tile([128, t], F16, tag=f"qk{j}", name=f"qkT{j}")
                      for j in range(4)]
            # v65_sb[tb][:, h, 0:64] = V head h rows tb; [:, h, 64] = 1.0
            v65_sb = [vp.tile([128, HPC, 65], F16, tag=f"v{tb}", name=f"v65_{tb}")
                      for tb in range(n_kb)]
            # normalized attention output^T: ot_sb[j] heads (2j, 2j+1)
            ot_sb = [otp.tile([128, t], F16, tag=f"ot{j}", name=f"ot{j}")
                     for j in range(2)]
            # transposed denominator staging: [128, 8] per head, so the
            # reciprocal runs over 8 elements/partition instead of 1024


            # ---- Stage A chunks: qk^T = W_qk @ x^T (+bias) ----
            a_done = [0]

            def emit_a(ob, tb2, pool):
                pa = pool.tile([128, 512], F32, tag="pab", name="pa")
                for cb in range(n_cb):
                    nc.tensor.matmul(
                        pa[:], wqk_sb[cb][:, ob * 128:(ob + 1) * 128],
                        xT_sb[cb][:, tb2 * 512:(tb2 + 1) * 512],
                        start=(cb == 0), stop=(cb == n_cb - 1))
                nc.vector.tensor_scalar_add(
                    qkT_sb[ob][:, tb2 * 512:(tb2 + 1) * 512],
                    pa[:], bqk_sb[:, ob:ob + 1])
                a_done[0] += 1

            # ---- Stage B chunks: V = x @ W_v^T ----
            b_done = [0]

            def emit_b(tb, pool):
                pt = pool.tile([128, 512], F32, tag="pab", name="pb")
                pb = pt[:, 0:CH]
                for cb in range(n_cb):
                    nc.tensor.matmul(
                        pb, xT_sb[cb][:, tb * 128:(tb + 1) * 128],
                        wv_sb[cb][:], start=(cb == 0), stop=(cb == n_cb - 1))
                nc.vector.memset(v65_sb[tb][:, :, 64:65], 1.0)
                nc.vector.tensor_copy(
                    v65_sb[tb][:, :, 0:64],
                    pb.rearrange("p (h d) -> p h d", d=D))
                b_done[0] += 1

            # serial prefix: q heads 0-1 and k heads 0-1 for the first 1024
            # tokens (everything the first scores tiles touch); the rest of
            # stage A and all of stage B interleave into the pipeline.
            n_pre = min(2, n_t2)   # 512-chunks covering the first q-chunk
            if phase >= 1:
                with tc.tile_pool(name="psPre", bufs=2, space="PSUM") as psPre:
                    emit_a(0, 0, psPre)
                    emit_a(2, 0, psPre)
                    if n_pre > 1:
                        emit_a(0, 1, psPre)
            fillers = deque()
            if phase >= 1:
                # pop order tuned so each chunk lands before its first use:
                # B(tb) needed at step tb+LAG of (qcc0,h0); k-chunks (ob2)
                # tb2=2,3 by steps 8/12; everything else has lots of slack.
                early = [("b", tb) for tb in range(min(4, n_kb))]
                if n_pre > 1:
                    early.insert(4, ("a", 2, 1))
                if n_t2 > 2:
                    early.append(("a", 2, 2))
                early += [("b", 4), ("b", 5)] if n_kb > 5 else []
                if n_t2 > 3:
                    early.append(("a", 2, 3))
                late = [("b", tb) for tb in range(6, n_kb)]
                late += [("a", 1, tb2) for tb2 in range(n_t2)]
                late += [("a", 3, tb2) for tb2 in range(n_t2)]
                late += [("a", 0, tb2) for tb2 in range(n_pre, n_t2)]
                late += [("a", 2, tb2) for tb2 in range(4, n_t2)]
                if phase < 2:
                    early = [it for it in early if it[0] != "b"]
                    late = [it for it in late if it[0] != "b"]
                for item in early + late:
                    if item[0] == "b" and item[1] >= n_kb:
                        continue
                    fillers.append(item)
            if (phase < 3 or SERIAL_AB) and fillers:
                with tc.tile_pool(name="psPre2", bufs=2, space="PSUM") as psPre2:
                    while fillers:
                        it = fillers.popleft()
                        if it[0] == "b":
                            emit_b(it[1], psPre2)
                        else:
                            emit_a(it[1], it[2], psPre2)

            n_qcc_eff = n_qcc if phase >= 3 else 0
            psY_ctx = ExitStack()
            psY = [None]

            # ---- Stage C: attention + output projection ----
            with tc.tile_pool(name="cTp", bufs=18) as cp, \
                 tc.tile_pool(name="ep", bufs=6) as ep, \
                 tc.tile_pool(name="pp", bufs=LAGQ + 3) as pp, \
                 tc.tile_pool(name="orp", bufs=3) as orp, \
                 tc.tile_pool(name="rbp", bufs=3) as rbp, \
                 tc.tile_pool(name="tmpp", bufs=3) as tmpp, \
                 tc.tile_pool(name="ysp", bufs=4) as ysp, \
                 tc.tile_pool(name="psS", bufs=2, space="PSUM") as psS, \
                 tc.tile_pool(name="psO", bufs=1, space="PSUM") as psO:

                psAB_ctx = ExitStack()
                psAB = [None]
                if fillers and n_qcc_eff:
                    psAB[0] = psAB_ctx.enter_context(
                        tc.tile_pool(name="psAB", bufs=2, space="PSUM"))

                def emit_outproj(qcc, ob, ns, use_scalar=False):
                    if psY[0] is None:
                        # psAB banks are free by now (all A/B chunks precede
                        # any out-projection in the filler queue)
                        psAB_ctx.close()
                        psY[0] = psY_ctx.enter_context(
                            tc.tile_pool(name="psY", bufs=2, space="PSUM"))
                    q0 = qcc * QW + ns * 512
                    Y = psY[0].tile([128, 512], F32, tag="Y", name="Y")
                    for cb in range(2):
                        nc.tensor.matmul(
                            Y[:], wo_sb[cb][:, ob * 128:(ob + 1) * 128],
                            ot_sb[cb][:, q0:q0 + 512],
                            start=(cb == 0), stop=(cb == 1))
                    ys = ysp.tile([128, 512], F16, tag="ys", name="ys")
                    if use_scalar:
                        nc.scalar.copy(ys[:], Y[:])
                    else:
                        nc.vector.tensor_copy(ys[:], Y[:])
                    nc.sync.dma_start(yT[ob * 128:(ob + 1) * 128, q0:q0 + 512],
                                      ys[:])

                def pop_fillers(budget, y_ok=True):
                    while fillers and budget > 0:
                        it = fillers[0]
                        if it[0] == "y" and not y_ok:
                            return
                        fillers.popleft()
                        if it[0] == "b":
                            emit_b(it[1], psAB[0])
                            budget -= 2048
                        elif it[0] == "a":
                            emit_a(it[1], it[2], psAB[0])
                            budget -= 4096
                        else:
                            emit_outproj(it[1], it[2], it[3])
                            budget = 0

                gstep = [0]
                pending = deque()   # (fn, is_last_of_head)

                def tick_attnv():
                    if pending and (len(pending) > LAGQ):
                        pending.popleft()()

                for qcc in range(n_qcc_eff):
                    q0 = qcc * QW
                    cT_tiles = []
                    for kb in range(n_kb):
                        ct = cp.tile([128, QW], F16, tag="ct", name=f"ct{kb}")
                        nc.sync.dma_start(ct[:],
                                          cT[kb * 128:(kb + 1) * 128, q0:q0 + QW])
                        cT_tiles.append(ct)

                    for h in range(HPC):
                        j, bp = h // 2, (h % 2) * 64
                        hs = {"O": None, "p": {}}

                        def emit_attnv(kb, hs=hs, h=h, j=j, bp=bp, qcc=qcc,
                                       q0=q0):
                            if kb == 0:
                                hs["O"] = psO.tile([65, QW], F32, tag="O",
                                                   name="O")
                            O = hs["O"]
                            P = hs["p"].pop(kb)
                            for ns in range(QW // 512):
                                nc.tensor.matmul(
                                    O[:, ns * 512:(ns + 1) * 512],
                                    v65_sb[kb][:, h, :],
                                    P[:, ns * 512:(ns + 1) * 512],
                                    start=(kb == 0), stop=(kb == n_kb - 1))
                            if kb != n_kb - 1:
                                return
                            # end of head: evacuate + normalize (the queue
                            # lag places these ops well past O's drain, so
                            # the in-order DVE stream never blocks on them)
                            den32 = rbp.tile([1, QW], F32, tag="den32",
                                             name="den32")
                            nc.vector.tensor_copy(den32[:], O[64:65, :])
                            oraw = orp.tile([64, QW], F16, tag="oraw",
                                            name="oraw")
                            nc.vector.tensor_copy(oraw[:], O[0:64, :])
                            rr32 = rbp.tile([1, QW], F32, tag="rr32",
                                            name="rr32")
                            nc.vector.reciprocal_approx_fast(rr32[:], den32[:])
                            rb32 = rbp.tile([64, QW], F32, tag="rb32",
                                            name="rb32")
                            nc.gpsimd.partition_broadcast(rb32[:], rr32[:])
                            tmp = tmpp.tile([64, QW], F16, tag="tmp",
                                            name="tmp")
                            nc.vector.tensor_mul(tmp[:], oraw[:], rb32[:])
                            nc.sync.dma_start(
                                ot_sb[j][bp:bp + 64, q0:q0 + QW], tmp[:])
                            if phase >= 5 and qcc < n_qcc - 1 and h == HPC - 1:
                                for ob in range(8):
                                    for ns in range(QW // 512):
                                        fillers.append(("y", qcc, ob, ns))

                        for i in range(n_kb):
                            if phase >= 4:
                                tick_attnv()
                            S = psS.tile([128, QW], F32, tag="S", name="S")
                            for ns in range(QW // 512):
                                nc.tensor.matmul(
                                    S[:, ns * 512:(ns + 1) * 512],
                                    qkT_sb[2 + j][bp:bp + 64, i * 128:(i + 1) * 128],
                                    qkT_sb[j][bp:bp + 64,
                                              q0 + ns * 512:q0 + (ns + 1) * 512],
                                    start=True, stop=True)
                            E = ep.tile([128, QW], F16, tag="E", name="E")
                            nc.scalar.activation(E[:], S[:], Exp, scale=1.0)
                            P = pp.tile([128, QW], F16, tag="P", name="P")
                            hs["p"][i] = P
                            use_pool = (PMODE == 1 or
                                        (PMODE == 2 and i % 2 == 1) or
                                        (PMODE == 3 and i % 4 == 3))
                            eng = nc.gpsimd if use_pool else nc.vector
                            eng.tensor_mul(P[:], E[:], cT_tiles[i][:])
                            if phase >= 4:
                                pending.append(
                                    lambda kb=i, fn=emit_attnv: fn(kb))
                            pop_fillers(FILLROWS,
                                        y_ok=(gstep[0] % 64 >= 40 or
                                              gstep[0] // 64 != qcc))
                            gstep[0] += 1

                # drain: remaining attnV queue, fillers, final out-proj
                while pending:
                    pending.popleft()()
                if n_qcc_eff:
                    while fillers:
                        pop_fillers(1 << 30)
                    if phase >= 5:
                        for ob in range(8):
                            for ns in range(QW // 512):
                                emit_outproj(n_qcc - 1, ob, ns,
                                             use_scalar=(ns == 0))
                psY_ctx.close()

    nc.compile()
    if compile_hw:
        nc.m = get_hw_module(nc.m)
    return nc


def make_in_maps(x, cutoffs, W_qkv, b_qkv, W_out):
    """Host-side sharding: slice + transpose + cast per core.

    wqkT column blocks: ob0 = Q heads {0,1}, ob1 = Q heads {2,3},
    ob2/ob3 = same for K; all scaled by sqrt(1/8)."""
    per_batch = []
    for b in range(B):
        xT_b = np.ascontiguousarray(x[b].T).astype(np.float16)
        cT_b = np.ascontiguousarray(cutoffs[b].T).astype(np.float16)
        per_batch.append((xT_b, cT_b))

    in_maps = []
    for core in range(NCORES):
        b, hg = core // HPC, core % HPC
        ch = slice(hg * CH, (hg + 1) * CH)
        chk = slice(DIM + hg * CH, DIM + (hg + 1) * CH)
        cols = [W_qkv[ch][:128], W_qkv[ch][128:],
                W_qkv[chk][:128], W_qkv[chk][128:]]
        wqkT_c = np.ascontiguousarray(
            (np.concatenate(cols, axis=0) * RSCALE).T).astype(np.float16)
        bv = np.concatenate([b_qkv[ch], b_qkv[chk]]) * RSCALE
        bqk_c = np.ascontiguousarray(
            np.stack([bv[0:128], bv[128:256], bv[256:384], bv[384:512]],
                     axis=1)).astype(np.float32)
        wvT_c = np.ascontiguousarray(
            W_qkv[2 * DIM + hg * CH:2 * DIM + (hg + 1) * CH, :].T).astype(np.float16)
        woT_c = np.ascontiguousarray(W_out[:, ch].T).astype(np.float16)
        in_maps.append({
            "xT": per_batch[b][0], "cT": per_batch[b][1],
            "wqkT": wqkT_c, "wvT": wvT_c, "woT": woT_c, "bqk": bqk_c,
        })
    return in_maps


def kernel(x, cutoffs, W_qkv, b_qkv, W_out, b_out):
    x = np.asarray(x, dtype=np.float32)
    cutoffs = np.asarray(cutoffs, dtype=np.float32)
    W_qkv = np.asarray(W_qkv, dtype=np.float32)
    b_qkv = np.asarray(b_qkv, dtype=np.float32)
    W_out = np.asarray(W_out, dtype=np.float32)
    b_out = np.asarray(b_out, dtype=np.float32)

    if "nc" not in _cache:
        _cache["nc"] = build_kernel()
    nc = _cache["nc"]

    in_maps = make_in_maps(x, cutoffs, W_qkv, b_qkv, W_out)
    res = None
    last_err = None
    for attempt in range(3):
        try:
            res = run_bass_kernel_spmd(nc, in_maps, core_ids=list(range(NCORES)),
                                       trace=False)
            break
        except Exception as e:  # transient NRT/axon failures: retry
            last_err = e
            import time
            time.sleep(5)
    if res is None:
        raise last_err

    y = np.zeros((B, T, DIM), dtype=np.float32)
    for core in range(NCORES):
        b = core // HPC
        y[b] += res.results[core]["yT"].astype(np.float32).T
    bias_vec = W_out @ b_qkv[2 * DIM:] + b_out
    y += bias_vec[None, None, :]
    return y


# revision 25
# speedup vs baseline: 2.0671x; 1.9897x over previous
"""Trainium2 Bass kernel for nn_Attention_2826088481156 (v2).

Dense transformer attention block:
    qkv = x @ W_qkv.T + b_qkv            [B,T,3,H,D]
    scores = q k^T * SCALE + log(clip(cutoffs, 1e-15))
    attn = softmax(scores)
    out  = (attn @ v) @ W_out.T + b_out

Sharding (8 NeuronCores): data-parallel over B (=2), tensor-parallel over
heads (16 heads -> 4 per core).  Each core computes the full attention for
its 4 heads and a partial output projection over its 256 channels; the
host sums the 4 partials per batch and adds the (host-folded) biases.

Key algebra on device:
    softmax(s + log c) = (c * exp(s)) / sum_k(c * exp(s))   [no log, no max]
    attn @ [V | 1] gives the weighted values and the softmax denominator
    (row 64 of the PSUM accumulator) in one pass.
    b_v and b_out never enter the nonlinearity; host folds y += W_out@b_v + b_out.
    sqrt(1/8) scaling folded into W_q/W_k/b_q/b_k host-side.

v2 structural changes vs the 388us v1 baseline (engine-level findings from
real NTFF traces):
  * ACT (exp over 16.8M scores/core) is the 141us floor; everything else
    must hide underneath it, and the PE must stay busy to hold its 2.4GHz
    p-state (idle gaps drop it to 1.2GHz - measured 427ns vs 216ns per
    512-col matmul).
  * P = E * cutoffs alternates between DVE and the otherwise-idle
    Pool/GPSIMD engine (DVE was 194us busy in v1; fp8 everywhere was
    tried and rejected: absmax error 2-6% vs the 2% budget).
  * v1 spent 52us in 8 single-partition DVE RECIPROCALs (6.4 cycles per
    element!); softmax now uses reciprocal_approx_fast (fp32, ~5x faster)
    per head, and the whole normalize chain is attached to the attnV
    queue so it lands in the engine streams well after its dependencies
    resolve (in-order engines otherwise block behind it).
  * attnV matmuls run through a cross-head queue LAGQ steps behind the
    scores, so every pipeline step is a uniform {1 attnV + 1 scores}
    pair with no per-head drain bursts; O PSUM is evacuated by a fast
    copy so psO runs with a single buffer.
  * stage B (V projection), most of stage A, and the qcc0 out-projection
    are emitted as deadline-tracked PE filler work inside the attention
    pipeline so the in-order PE stream always has independent work while
    ACT chews (keeps the p-state up AND hides ~60us of projection work).
  * measured same-session A/B vs v1: 361us -> 291us per rep.
"""

import numpy as np

import concourse.bass as bass
import concourse.tile as tile
from concourse import bacc, mybir
from concourse.bass_utils import run_bass_kernel_spmd
from concourse.bass_interp import get_hw_module

F16 = mybir.dt.float16
F32 = mybir.dt.float32
Exp = mybir.ActivationFunctionType.Exp

DIM = 1024
H = 16
D = 64
B = 2
T = 2048
RSCALE = 0.125 ** 0.5   # folded into W_q, W_k, b_q, b_k on host
HPC = 4                 # heads per core
CH = HPC * D            # 256 channels per core
NCORES = 8

_cache = {}


def build_kernel(t=T, compile_hw=True, loop_reps=0, phase=5, opt=None):
    import os
    from collections import deque
    from contextlib import ExitStack, nullcontext
    _opt = dict(opt or {})
    for k in ("LAGQ", "FILLROWS", "SERIAL_AB", "PMODE"):
        if f"K_{k}" in os.environ:
            _opt.setdefault(k, int(os.environ[f"K_{k}"]))
    LAGQ = int(_opt.get("LAGQ", 10))     # attnV trails scores by LAGQ steps
    FILLROWS = int(_opt.get("FILLROWS", 2560))  # PE filler rows per step
    SERIAL_AB = int(_opt.get("SERIAL_AB", 0))
    PMODE = int(_opt.get("PMODE", 3))    # P-mult: 0=DVE, 1=Pool, 2=50/50, 3=25% Pool

    nc = bacc.Bacc("TRN2", target_bir_lowering=False, debug=False,
                   num_devices=NCORES)

    n_cb = DIM // 128           # 8 contraction blocks for projections
    n_kb = t // 128             # 16 key blocks
    QW = 1024 if t >= 1024 else t
    n_qcc = t // QW
    n_t2 = t // 512             # 512-token chunks for stage A

    xT = nc.dram_tensor("xT", [DIM, t], F16, kind="ExternalInput")
    cT = nc.dram_tensor("cT", [t, t], F16, kind="ExternalInput")
    wqkT = nc.dram_tensor("wqkT", [DIM, 512], F16, kind="ExternalInput")
    wvT = nc.dram_tensor("wvT", [DIM, CH], F16, kind="ExternalInput")
    woT = nc.dram_tensor("woT", [CH, DIM], F16, kind="ExternalInput")
    bqk = nc.dram_tensor("bqk", [128, 4], F32, kind="ExternalInput")
    yT = nc.dram_tensor("yT", [DIM, t], F16, kind="ExternalOutput")

    with tile.TileContext(nc) as tc:
        loop_ctx = tc.For_i(0, loop_reps, 1) if loop_reps else nullcontext()
        with loop_ctx, ExitStack() as ctx:
            const = ctx.enter_context(tc.tile_pool(name="const", bufs=1))
            qkp = ctx.enter_context(tc.tile_pool(name="qkT", bufs=1))
            vp = ctx.enter_context(tc.tile_pool(name="v65", bufs=1))
            otp = ctx.enter_context(tc.tile_pool(name="ot", bufs=1))
            xp = ctx.enter_context(tc.tile_pool(name="xTp", bufs=1))

            wqk_sb = []
            for cb in range(n_cb):
                w1 = const.tile([128, 512], F16, tag=f"wqk{cb}", name=f"wqk{cb}")
                nc.sync.dma_start(w1[:], wqkT[cb * 128:(cb + 1) * 128, :])
                wqk_sb.append(w1)
            xT_sb = [xp.tile([128, t], F16, tag=f"x{cb}", name=f"xt{cb}")
                     for cb in range(n_cb)]
            for tb2 in range(n_t2):
                for cb in range(n_cb):
                    nc.sync.dma_start(
                        xT_sb[cb][:, tb2 * 512:(tb2 + 1) * 512],
                        xT[cb * 128:(cb + 1) * 128, tb2 * 512:(tb2 + 1) * 512])
            wv_sb = []
            for cb in range(n_cb):
                w2 = const.tile([128, CH], F16, tag=f"wv{cb}", name=f"wv{cb}")
                nc.sync.dma_start(w2[:], wvT[cb * 128:(cb + 1) * 128, :])
                wv_sb.append(w2)
            wo_sb = []
            for j in range(2):
                w3 = const.tile([128, DIM], F16, tag=f"wo{j}", name=f"wo{j}")
                nc.sync.dma_start(w3[:], woT[j * 128:(j + 1) * 128, :])
                wo_sb.append(w3)
            bqk_sb = const.tile([128, 4], F32, tag="bqk")
            nc.sync.dma_start(bqk_sb[:], bqk[:, :])

            # qkT_sb[j]: j=0 Q heads 0-1, j=1 Q heads 2-3, j=2/3 same for K
            qkT_sb = [qkp.tile([128, t], F16, tag=f"qk{j}", name=f"qkT{j}")
                      for j in range(4)]
            # v65_sb[tb][:, h, 0:64] = V head h rows tb; [:, h, 64] = 1.0
            v65_sb = [vp.tile([128, HPC, 65], F16, tag=f"v{tb}", name=f"v65_{tb}")
                      for tb in range(n_kb)]
            # normalized attention output^T: ot_sb[j] heads (2j, 2j+1)
            ot_sb = [otp.tile([128, t], F16, tag=f"ot{j}", name=f"ot{j}")
                     for j in range(2)]
            # ---- Stage A chunks: qk^T = W_qk @ x^T (+bias) ----
            a_done = [0]

            def emit_a(ob, tb2, pool):
                pa = pool.tile([128, 512], F32, tag="pab", name="pa")
                for cb in range(n_cb):
                    nc.tensor.matmul(
                        pa[:], wqk_sb[cb][:, ob * 128:(ob + 1) * 128],
                        xT_sb[cb][:, tb2 * 512:(tb2 + 1) * 512],
                        start=(cb == 0), stop=(cb == n_cb - 1))
                nc.vector.tensor_scalar_add(
                    qkT_sb[ob][:, tb2 * 512:(tb2 + 1) * 512],
                    pa[:], bqk_sb[:, ob:ob + 1])
                a_done[0] += 1

            # ---- Stage B chunks: V = x @ W_v^T ----
            b_done = [0]

            def emit_b(tb, pool):
                pt = pool.tile([128, 512], F32, tag="pab", name="pb")
                pb = pt[:, 0:CH]
                for cb in range(n_cb):
                    nc.tensor.matmul(
                        pb, xT_sb[cb][:, tb * 128:(tb + 1) * 128],
                        wv_sb[cb][:], start=(cb == 0), stop=(cb == n_cb - 1))
                nc.vector.memset(v65_sb[tb][:, :, 64:65], 1.0)
                nc.vector.tensor_copy(
                    v65_sb[tb][:, :, 0:64],
                    pb.rearrange("p (h d) -> p h d", d=D))
                b_done[0] += 1

            # serial prefix: q heads 0-1 and k heads 0-1 for the first 1024
            # tokens (everything the first scores tiles touch); the rest of
            # stage A and all of stage B interleave into the pipeline.
            n_pre = min(2, n_t2)   # 512-chunks covering the first q-chunk
            if phase >= 1:
                with tc.tile_pool(name="psPre", bufs=2, space="PSUM") as psPre:
                    emit_a(0, 0, psPre)
                    emit_a(2, 0, psPre)
                    if n_pre > 1:
                        emit_a(0, 1, psPre)
            fillers = deque()
            if phase >= 1:
                # pop order tuned so each chunk lands before its first use:
                # B(tb) needed at step tb+LAG of (qcc0,h0); k-chunks (ob2)
                # tb2=2,3 by steps 8/12; everything else has lots of slack.
                early = [("b", tb) for tb in range(min(4, n_kb))]
                if n_pre > 1:
                    early.insert(4, ("a", 2, 1))
                if n_t2 > 2:
                    early.append(("a", 2, 2))
                early += [("b", 4), ("b", 5)] if n_kb > 5 else []
                if n_t2 > 3:
                    early.append(("a", 2, 3))
                late = [("b", tb) for tb in range(6, n_kb)]
                late += [("a", 1, tb2) for tb2 in range(n_t2)]
                late += [("a", 3, tb2) for tb2 in range(n_t2)]
                late += [("a", 0, tb2) for tb2 in range(n_pre, n_t2)]
                late += [("a", 2, tb2) for tb2 in range(4, n_t2)]
                if phase < 2:
                    early = [it for it in early if it[0] != "b"]
                    late = [it for it in late if it[0] != "b"]
                for item in early + late:
                    if item[0] == "b" and item[1] >= n_kb:
                        continue
                    fillers.append(item)
            if (phase < 3 or SERIAL_AB) and fillers:
                with tc.tile_pool(name="psPre2", bufs=2, space="PSUM") as psPre2:
                    while fillers:
                        it = fillers.popleft()
                        if it[0] == "b":
                            emit_b(it[1], psPre2)
                        else:
                            emit_a(it[1], it[2], psPre2)

            n_qcc_eff = n_qcc if phase >= 3 else 0
            psY_ctx = ExitStack()
            psY = [None]

            # ---- Stage C: attention + output projection ----
            with tc.tile_pool(name="cTp", bufs=18) as cp, \
                 tc.tile_pool(name="ep", bufs=6) as ep, \
                 tc.tile_pool(name="pp", bufs=LAGQ + 3) as pp, \
                 tc.tile_pool(name="orp", bufs=3) as orp, \
                 tc.tile_pool(name="rbp", bufs=3) as rbp, \
                 tc.tile_pool(name="tmpp", bufs=3) as tmpp, \
                 tc.tile_pool(name="ysp", bufs=4) as ysp, \
                 tc.tile_pool(name="psS", bufs=2, space="PSUM") as psS, \
                 tc.tile_pool(name="psO", bufs=1, space="PSUM") as psO:

                psAB_ctx = ExitStack()
                psAB = [None]
                if fillers and n_qcc_eff:
                    psAB[0] = psAB_ctx.enter_context(
                        tc.tile_pool(name="psAB", bufs=2, space="PSUM"))

                def emit_outproj(qcc, ob, ns, use_scalar=False):
                    if psY[0] is None:
                        # psAB banks are free by now (all A/B chunks precede
                        # any out-projection in the filler queue)
                        psAB_ctx.close()
                        psY[0] = psY_ctx.enter_context(
                            tc.tile_pool(name="psY", bufs=2, space="PSUM"))
                    q0 = qcc * QW + ns * 512
                    Y = psY[0].tile([128, 512], F32, tag="Y", name="Y")
                    for cb in range(2):
                        nc.tensor.matmul(
                            Y[:], wo_sb[cb][:, ob * 128:(ob + 1) * 128],
                            ot_sb[cb][:, q0:q0 + 512],
                            start=(cb == 0), stop=(cb == 1))
                    ys = ysp.tile([128, 512], F16, tag="ys", name="ys")
                    if use_scalar:
                        nc.scalar.copy(ys[:], Y[:])
                    else:
                        nc.vector.tensor_copy(ys[:], Y[:])
                    nc.sync.dma_start(yT[ob * 128:(ob + 1) * 128, q0:q0 + 512],
                                      ys[:])

                def pop_fillers(budget, y_ok=True):
                    while fillers and budget > 0:
                        it = fillers[0]
                        if it[0] == "y" and not y_ok:
                            return
                        fillers.popleft()
                        if it[0] == "b":
                            emit_b(it[1], psAB[0])
                            budget -= 2048
                        elif it[0] == "a":
                            emit_a(it[1], it[2], psAB[0])
                            budget -= 4096
                        else:
                            emit_outproj(it[1], it[2], it[3])
                            budget = 0

                gstep = [0]
                pending = deque()   # (fn, is_last_of_head)

                def tick_attnv():
                    if pending and (len(pending) > LAGQ):
                        pending.popleft()()

                for qcc in range(n_qcc_eff):
                    q0 = qcc * QW
                    cT_tiles = []
                    for kb in range(n_kb):
                        ct = cp.tile([128, QW], F16, tag="ct", name=f"ct{kb}")
                        nc.sync.dma_start(ct[:],
                                          cT[kb * 128:(kb + 1) * 128, q0:q0 + QW])
                        cT_tiles.append(ct)

                    for h in range(HPC):
                        j, bp = h // 2, (h % 2) * 64
                        hs = {"O": None, "p": {}}

                        def emit_attnv(kb, hs=hs, h=h, j=j, bp=bp, qcc=qcc,
                                       q0=q0):
                            if kb == 0:
                                hs["O"] = psO.tile([65, QW], F32, tag="O",
                                                   name="O")
                            O = hs["O"]
                            P = hs["p"].pop(kb)
                            for ns in range(QW // 512):
                                nc.tensor.matmul(
                                    O[:, ns * 512:(ns + 1) * 512],
                                    v65_sb[kb][:, h, :],
                                    P[:, ns * 512:(ns + 1) * 512],
                                    start=(kb == 0), stop=(kb == n_kb - 1))
                            if kb != n_kb - 1:
                                return
                            # end of head: evacuate + normalize (the queue
                            # lag places these ops well past O's drain, so
                            # the in-order DVE stream never blocks on them)
                            den32 = rbp.tile([1, QW], F32, tag="den32",
                                             name="den32")
                            nc.vector.tensor_copy(den32[:], O[64:65, :])
                            oraw = orp.tile([64, QW], F16, tag="oraw",
                                            name="oraw")
                            nc.vector.tensor_copy(oraw[:], O[0:64, :])
                            rr32 = rbp.tile([1, QW], F32, tag="rr32",
                                            name="rr32")
                            nc.vector.reciprocal_approx_fast(rr32[:], den32[:])
                            rb32 = rbp.tile([64, QW], F32, tag="rb32",
                                            name="rb32")
                            nc.gpsimd.partition_broadcast(rb32[:], rr32[:])
                            tmp = tmpp.tile([64, QW], F16, tag="tmp",
                                            name="tmp")
                            nc.vector.tensor_mul(tmp[:], oraw[:], rb32[:])
                            nc.sync.dma_start(
                                ot_sb[j][bp:bp + 64, q0:q0 + QW], tmp[:])
                            if phase >= 5 and qcc < n_qcc - 1 and h == HPC - 1:
                                for ob in range(8):
                                    for ns in range(QW // 512):
                                        fillers.append(("y", qcc, ob, ns))

                        for i in range(n_kb):
                            if phase >= 4:
                                tick_attnv()
                            S = psS.tile([128, QW], F32, tag="S", name="S")
                            for ns in range(QW // 512):
                                nc.tensor.matmul(
                                    S[:, ns * 512:(ns + 1) * 512],
                                    qkT_sb[2 + j][bp:bp + 64, i * 128:(i + 1) * 128],
                                    qkT_sb[j][bp:bp + 64,
                                              q0 + ns * 512:q0 + (ns + 1) * 512],
                                    start=True, stop=True)
                            E = ep.tile([128, QW], F16, tag="E", name="E")
                            nc.scalar.activation(E[:], S[:], Exp, scale=1.0)
                            P = pp.tile([128, QW], F16, tag="P", name="P")
                            hs["p"][i] = P
                            use_pool = (PMODE == 1 or
                                        (PMODE == 2 and i % 2 == 1) or
                                        (PMODE == 3 and i % 4 == 3))
                            eng = nc.gpsimd if use_pool else nc.vector
                            eng.tensor_mul(P[:], E[:], cT_tiles[i][:])
                            if phase >= 4:
                                pending.append(
                                    lambda kb=i, fn=emit_attnv: fn(kb))
                            pop_fillers(FILLROWS,
                                        y_ok=(gstep[0] % 64 >= 40 or
                                              gstep[0] // 64 != qcc))
                            gstep[0] += 1

                # drain: remaining attnV queue, fillers, final out-proj
                while pending:
                    pending.popleft()()
                if n_qcc_eff:
                    while fillers:
                        pop_fillers(1 << 30)
                    if phase >= 5:
                        for ob in range(8):
                            for ns in range(QW // 512):
                                emit_outproj(n_qcc - 1, ob, ns,
                                             use_scalar=(ns == 0))
                psY_ctx.close()

    nc.compile()
    if compile_hw:
        nc.m = get_hw_module(nc.m)
    return nc


def make_in_maps(x, cutoffs, W_qkv, b_qkv, W_out):
    """Host-side sharding: slice + transpose + cast per core.

    wqkT column blocks: ob0 = Q heads {0,1}, ob1 = Q heads {2,3},
    ob2/ob3 = same for K; all scaled by sqrt(1/8)."""
    per_batch = []
    for b in range(B):
        xT_b = np.ascontiguousarray(x[b].T).astype(np.float16)
        cT_b = np.ascontiguousarray(cutoffs[b].T).astype(np.float16)
        per_batch.append((xT_b, cT_b))

    in_maps = []
    for core in range(NCORES):
        b, hg = core // HPC, core % HPC
        ch = slice(hg * CH, (hg + 1) * CH)
        chk = slice(DIM + hg * CH, DIM + (hg + 1) * CH)
        cols = [W_qkv[ch][:128], W_qkv[ch][128:],
                W_qkv[chk][:128], W_qkv[chk][128:]]
        wqkT_c = np.ascontiguousarray(
            (np.concatenate(cols, axis=0) * RSCALE).T).astype(np.float16)
        bv = np.concatenate([b_qkv[ch], b_qkv[chk]]) * RSCALE
        bqk_c = np.ascontiguousarray(
            np.stack([bv[0:128], bv[128:256], bv[256:384], bv[384:512]],
                     axis=1)).astype(np.float32)
        wvT_c = np.ascontiguousarray(
            W_qkv[2 * DIM + hg * CH:2 * DIM + (hg + 1) * CH, :].T).astype(np.float16)
        woT_c = np.ascontiguousarray(W_out[:, ch].T).astype(np.float16)
        in_maps.append({
            "xT": per_batch[b][0], "cT": per_batch[b][1],
            "wqkT": wqkT_c, "wvT": wvT_c, "woT": woT_c, "bqk": bqk_c,
        })
    return in_maps


def kernel(x, cutoffs, W_qkv, b_qkv, W_out, b_out):
    x = np.asarray(x, dtype=np.float32)
    cutoffs = np.asarray(cutoffs, dtype=np.float32)
    W_qkv = np.asarray(W_qkv, dtype=np.float32)
    b_qkv = np.asarray(b_qkv, dtype=np.float32)
    W_out = np.asarray(W_out, dtype=np.float32)
    b_out = np.asarray(b_out, dtype=np.float32)

    if "nc" not in _cache:
        _cache["nc"] = build_kernel()
    nc = _cache["nc"]

    in_maps = make_in_maps(x, cutoffs, W_qkv, b_qkv, W_out)
    res = None
    last_err = None
    for attempt in range(3):
        try:
            res = run_bass_kernel_spmd(nc, in_maps, core_ids=list(range(NCORES)),
                                       trace=False)
            break
        except Exception as e:  # transient NRT/axon failures: retry
            last_err = e
            import time
            time.sleep(5)
    if res is None:
        raise last_err

    y = np.zeros((B, T, DIM), dtype=np.float32)
    for core in range(NCORES):
        b = core // HPC
        y[b] += res.results[core]["yT"].astype(np.float32).T
    bias_vec = W_out @ b_qkv[2 * DIM:] + b_out
    y += bias_vec[None, None, :]
    return y
